# revision 1
# baseline (speedup 1.0000x reference)
"""ALiBi multi-head attention on 8 TRN2 NeuronCores.

Problem: B=2, S=2048, E=1024, H=16 heads of D=64, fp32.
  q/k/v = inputs @ W* + b*;  scores = q k^T / sqrt(D) + slope_h * (j - i)
  out = softmax(scores) @ v, heads concat, @ Wo + bo.

Sharding: tensor-parallel over heads — 2 heads per core, both batches on
every core. Each core computes its heads' q/k/v projections, attention,
and a partial output projection (row-parallel Wo); the host sums the 8
partials and adds bo (the standard row-parallel unshard).

Key algebra: softmax over j is invariant to per-row shifts, so
  softmax(qk*scale + slope*(j-i)) == softmax(qk*scale + slope*j - C)
for any per-row constant C. We factor exp(qk*scale + slope*j - C) as
  exp(qk*scale + slope*(j%128) - C_h)  *  exp(slope*128*(jc - 15))
where jc = j//128. The first factor's bias is per-PSUM-partition and
jc-independent — ONE Exp activation bias column per head, enabling wide
multi-j-chunk Exp ops. The second factor is folded into v (and into the
all-ones ride-along column appended to v that accumulates the softmax
denominators through the same PV matmul), scaled per token-chunk at v
build time. This kills the (j-i) bias matrix, the row-max pass, and the
row-sum pass entirely.

The light head slot ("B", heads 0-7) only processes the last 3 j-chunks:
the ALiBi decay bounds every dropped weight below e^-16 of the row sum.
The heavy slot ("A", heads 8-15) runs all 16. Pairing one A head with
one B head per core keeps the SPMD instruction stream identical on all
8 cores while cutting attention work ~40%.

Everything is computed in transposed orientation from a host-precomputed
X^T, so no on-device transposes are needed except v (32 small PE
transposes). Matmul operands are bf16 (f32 PSUM accumulate); k^T is
stored zero-padded per head slot so every matmul contracts K=128 and the
PE never switches tiling mode. Rel err vs the f32 reference is ~4e-3
(dominated by bf16 operand rounding), well inside the 2e-2 gate.
"""

import numpy as np

NUM_HEADS = 16
E = 1024
D = 64
B = 2
S = 2048
N_CORES = 8
HL = NUM_HEADS // N_CORES      # heads per core = 2
COLS = HL * D                  # per-core projection width = 128
NT = B * S                     # total tokens = 4096
MARGIN = 8.0
JG = 2                         # j-chunks per Exp group

_CACHE = {}


def _alibi_slopes():
    x = (2.0 ** 8) ** (1.0 / NUM_HEADS)
    return [1.0 / x ** (i + 1) for i in range(NUM_HEADS)]


def build_nc():
    import concourse.mybir as mybir
    import concourse.tile as tile
    from concourse import bacc
    from concourse.masks import make_identity

    f32 = mybir.dt.float32
    bf16 = mybir.dt.bfloat16
    Exp = mybir.ActivationFunctionType.Exp

    nc = bacc.Bacc("TRN2", target_bir_lowering=False, debug=False,
                   num_devices=N_CORES)

    xt_ext = nc.declare_dram_parameter("xt", [E, NT], bf16, isOutput=False)
    wq_ext = nc.declare_dram_parameter("wq", [E, COLS], bf16, isOutput=False)
    wk_ext = nc.declare_dram_parameter("wk", [E, COLS], bf16, isOutput=False)
    wv_ext = nc.declare_dram_parameter("wv", [E, COLS], bf16, isOutput=False)
    bqkv_ext = nc.declare_dram_parameter("bqkv", [COLS, 3], f32, isOutput=False)
    wo_ext = nc.declare_dram_parameter("wo", [COLS, E], bf16, isOutput=False)
    bias_ext = nc.declare_dram_parameter("bias", [128, HL], f32, isOutput=False)
    onesv_ext = nc.declare_dram_parameter("onesv", [128, NT // 128, 2], bf16,
                                          isOutput=False)
    vscale_ext = nc.declare_dram_parameter("vscale", [128, NT // 128, 2], f32,
                                           isOutput=False)
    out_ext = nc.declare_dram_parameter("out", [NT, E], f32, isOutput=True)

    NTB = NT // 512            # 8 token blocks for projections
    NJC = S // 128             # 16 j-chunks per batch
    NG = NJC // JG             # exp groups per (b, qi, head)
    NQI = S // 512             # 4 qi-blocks per batch
    NTC = NT // 128            # 32 global token chunks

    from contextlib import ExitStack
    with tile.TileContext(nc) as tc, ExitStack() as stack:
        with (
            tc.tile_pool(name="persist", bufs=1) as pp,
            tc.tile_pool(name="stage", bufs=2) as stp,
            tc.tile_pool(name="exp", bufs=4) as expp,
            tc.tile_pool(name="norm", bufs=3) as nrm,
            tc.tile_pool(name="ctx", bufs=3) as ctxp,
            tc.tile_pool(name="outp", bufs=4) as outp,
        ):
            # ---- constants ----
            wq_sb = pp.tile([128, E], bf16, tag="wq")
            wk_sb = pp.tile([128, E], bf16, tag="wk")
            wv_sb = pp.tile([128, E], bf16, tag="wv")
            xt_full = pp.tile([128, 8, NT], bf16, tag="xt_full")
            for kc in range(8):
                nc.sync.dma_start(out=wq_sb[:, kc * 128:(kc + 1) * 128],
                                  in_=wq_ext[kc * 128:(kc + 1) * 128, :])
                nc.sync.dma_start(out=xt_full[:, kc, 0:512],
                                  in_=xt_ext[kc * 128:(kc + 1) * 128, 0:512])
            for w_sb, w_ext in ((wk_sb, wk_ext), (wv_sb, wv_ext)):
                for kc in range(8):
                    nc.sync.dma_start(
                        out=w_sb[:, kc * 128:(kc + 1) * 128],
                        in_=w_ext[kc * 128:(kc + 1) * 128, :])
            bqkv_sb = pp.tile([128, 3], f32, tag="bqkv")
            nc.sync.dma_start(out=bqkv_sb[:], in_=bqkv_ext[:])
            wo_sb = pp.tile([128, E], bf16, tag="wo")
            nc.sync.dma_start(out=wo_sb[:], in_=wo_ext[:])
            bias_sb = pp.tile([128, HL], f32, tag="bias")
            nc.sync.dma_start(out=bias_sb[:], in_=bias_ext[:])
            vscale_sb = pp.tile([128, NTC, 2], f32, tag="vscale")
            nc.sync.dma_start(out=vscale_sb[:], in_=vscale_ext[:])
            ident = pp.tile([128, 128], bf16, tag="ident")
            make_identity(nc, ident[:])

            qT = pp.tile([128, NT], bf16, tag="qT")
            # kT split per head slot, zero-padded on the other slot's rows:
            # keeps every matmul in 128-row mode (no PE tiling-mode switches)
            kTA = pp.tile([128, NT], bf16, tag="kTA")
            kTB = pp.tile([128, NT], bf16, tag="kTB")
            nc.vector.memset(kTA[D:2 * D, :], 0.0)
            nc.vector.memset(kTB[0:D, :], 0.0)
            v_sb = pp.tile([128, NTC, 2 * (D + 1)], bf16, tag="v")
            # scaled "ones" columns (64, 129): the row-sum ride-along,
            # pre-multiplied by the per-chunk ALiBi factor exp(slope*128*(jc-15))
            sv_stage = pp.tile([128, NTC, 2], bf16, tag="sv_stage")
            nc.sync.dma_start(out=sv_stage[:], in_=onesv_ext[:])
            nc.vector.tensor_copy(v_sb[:, :, D:D + 1], sv_stage[:, :, 0:1])
            nc.vector.tensor_copy(v_sb[:, :, 2 * D + 1:2 * D + 2],
                                  sv_stage[:, :, 1:2])

            psQK = stack.enter_context(
                tc.tile_pool(name="psQK", bufs=2, space="PSUM"))
            with (
                tc.tile_pool(name="psA", bufs=2, space="PSUM") as psA,
                tc.tile_pool(name="psT", bufs=2, space="PSUM") as psT,
            ):
                with nc.named_scope("proj"):
                    for tb in range(NTB):
                        ts = slice(tb * 512, (tb + 1) * 512)
                        if tb > 0:
                            for kc in range(8):
                                nc.sync.dma_start(
                                    out=xt_full[:, kc, ts],
                                    in_=xt_ext[kc * 128:(kc + 1) * 128, ts])
                        for pi, (w_sb, dst) in enumerate(
                            ((wq_sb, qT), (wk_sb, "k"), (wv_sb, None))
                        ):
                            ps = psA.tile([128, 512], f32, tag="pa", name="ps")
                            for kc in range(8):
                                nc.tensor.matmul(
                                    ps[:],
                                    w_sb[:, kc * 128:(kc + 1) * 128],
                                    xt_full[:, kc, ts],
                                    start=(kc == 0), stop=(kc == 7))
                            if dst is qT:
                                nc.vector.tensor_scalar_add(
                                    dst[:, ts], ps[:], bqkv_sb[:, pi:pi + 1])
                            elif dst == "k":
                                nc.vector.tensor_scalar_add(
                                    kTA[0:D, ts], ps[0:D, :],
                                    bqkv_sb[0:D, pi:pi + 1])
                                nc.vector.tensor_scalar_add(
                                    kTB[D:2 * D, ts], ps[D:2 * D, :],
                                    bqkv_sb[D:2 * D, pi:pi + 1])
                            else:
                                vT_t = stp.tile([128, 512], bf16, tag="vT")
                                nc.vector.tensor_scalar_add(
                                    vT_t[:], ps[:], bqkv_sb[:, pi:pi + 1])
                                for i in range(4):
                                    t = tb * 4 + i
                                    pt = psT.tile([128, 128], bf16, tag="pt", name="pt")
                                    nc.tensor.transpose(
                                        pt[:], vT_t[:, i * 128:(i + 1) * 128],
                                        ident[:])
                                    # scale v rows by the per-chunk ALiBi factor
                                    nc.vector.tensor_scalar_mul(
                                        v_sb[:, t, 0:D],
                                        pt[:, 0:D], vscale_sb[:, t, 0:1])
                                    nc.vector.tensor_scalar_mul(
                                        v_sb[:, t, D + 1:2 * D + 1],
                                        pt[:, D:2 * D], vscale_sb[:, t, 1:2])

            with (
                tc.tile_pool(name="psC", bufs=1, space="PSUM") as psC,
                tc.tile_pool(name="psW", bufs=2, space="PSUM") as psW,
            ):
                def emit_wo(ctx_sb, boff, qi):
                    for tc4 in range(4):
                        o_t = outp.tile([128, E], f32, tag="out", name="o_t")
                        for ec in range(2):
                            wo_ps = psW.tile([128, 512], f32, tag="wo",
                                             name="wo_ps")
                            nc.tensor.matmul(
                                wo_ps[:],
                                ctx_sb[:, tc4 * 128:(tc4 + 1) * 128],
                                wo_sb[:, ec * 512:(ec + 1) * 512],
                                start=True, stop=True)
                            nc.vector.tensor_copy(
                                o_t[:, ec * 512:(ec + 1) * 512], wo_ps[:])
                        r0 = boff + qi * 512 + tc4 * 128
                        nc.sync.dma_start(out=out_ext[r0:r0 + 128, :], in_=o_t[:])

                with nc.named_scope("attn"):
                    for b in range(B):
                        boff = b * S
                        for qi in range(NQI):
                            qs = slice(boff + qi * 512, boff + qi * 512 + 512)
                            ctx_ps = [psC.tile([D + 1, 512], f32, tag=f"ctx{h}",
                                              name=f"ctx{h}")
                                      for h in range(HL)]
                            ctx_sb = ctxp.tile([128, 512], bf16, tag="ctx_sb")

                            def emit_norm(h):
                                # drain the ctx PSUM bank (sums row + ctx rows)
                                # then normalize into ctx_sb
                                s_t = nrm.tile([1, 512], f32, tag=f"s{h}",
                                               name=f"s{h}")
                                nc.scalar.copy(s_t[:], ctx_ps[h][D:D + 1, :])
                                ctxc = nrm.tile([D, 512], f32, tag=f"cc{h}",
                                                name=f"cc{h}")
                                nc.scalar.copy(ctxc[:], ctx_ps[h][0:D, :])
                                sb_t = nrm.tile([D, 512], f32, tag="sb")
                                nc.gpsimd.partition_broadcast(sb_t[:], s_t[:])
                                r_t = nrm.tile([D, 512], f32, tag="r")
                                nc.vector.reciprocal_approx_fast(r_t[:], sb_t[:])
                                nc.vector.tensor_mul(
                                    ctx_sb[h * D:(h + 1) * D, :],
                                    ctxc[:], r_t[:])
                            # slot 0 ("A" = heavy head, all 16 j-chunks) and
                            # slot 1 ("B" = light head, last 4 j-chunks only --
                            # ALiBi decay makes earlier chunks' weights < e^-24)
                            # (head-slot, first j-chunk, #chunks): A = all 16,
                            # B = last 3 (ALiBi decay bounds the rest < e^-16).
                            # B runs early so its normalization hides under A's
                            # remaining groups; only A's norm sits at the unit
                            # boundary.
                            sched = ([(0, g * JG, JG) for g in range(2)]
                                     + [(1, NJC - 3, 2), (1, NJC - 1, 1)]
                                     + [(0, g * JG, JG) for g in range(2, NG)])
                            for h, jc0, w in sched:
                                kTh = kTA if h == 0 else kTB
                                hc = slice(h * (D + 1), (h + 1) * (D + 1))
                                jc_lo = 0 if h == 0 else NJC - 3
                                qk_t = psQK.tile([128, JG, 512], f32,
                                                 tag="qk", name="qk")
                                for u in range(w):
                                    jc = jc0 + u
                                    j0 = boff + jc * 128
                                    nc.tensor.matmul(
                                        qk_t[:, u, :],
                                        kTh[:, j0:j0 + 128],
                                        qT[:, qs],
                                        start=True, stop=True)
                                e_t = expp.tile([128, JG, 512], bf16, tag="exp")
                                nc.scalar.activation(
                                    e_t[:, 0:w, :], qk_t[:, 0:w, :], Exp,
                                    bias=bias_sb[:, h:h + 1], scale=1.0)
                                done = False
                                for u in range(w):
                                    jc = jc0 + u
                                    t = b * NJC + jc
                                    nc.tensor.matmul(
                                        ctx_ps[h][:],
                                        v_sb[:, t, hc],
                                        e_t[:, u, :],
                                        start=(jc == jc_lo), stop=(jc == NJC - 1))
                                    done = done or (jc == NJC - 1)
                                if done:
                                    emit_norm(h)

                            emit_wo(ctx_sb, boff, qi)
    nc.compile()
    return nc


def _prepare_in_maps(inputs, Wq, bq, Wk, bk, Wv, bv, Wo, bo):
    import ml_dtypes
    f32 = np.float32
    bf = ml_dtypes.bfloat16
    X = np.asarray(inputs, dtype=f32).reshape(NT, E)
    xt = np.ascontiguousarray(X.T).astype(bf)
    slopes = _alibi_slopes()
    scale = 1.0 / np.sqrt(D)
    NTC = NT // 128
    NJC = S // 128
    in_maps = []
    for c in range(N_CORES):
        # slot 0 = heavy head (small slope, all chunks), slot 1 = light head
        heads = (8 + c, c)
        cols = np.concatenate([np.arange(h * D, (h + 1) * D) for h in heads])
        bqkv = np.stack([bq[cols] * scale, bk[cols], bv[cols]], axis=1)
        bias_c = np.zeros((128, HL), dtype=f32)
        onesv = np.zeros((128, NTC, 2), dtype=f32)  # cast to bf16 below
        vscale = np.zeros((128, NTC, 2), dtype=f32)
        p = np.arange(128)
        for l, hh in enumerate(heads):
            sl = slopes[hh]
            bias_c[:, l] = sl * p - sl * 127.0 - MARGIN
            for t in range(NTC):
                jc = t % NJC
                f = np.exp(sl * 128.0 * (jc - (NJC - 1)), dtype=np.float64)
                onesv[:, t, l] = f
                vscale[:, t, l] = f
        in_maps.append({
            "xt": xt,
            "wq": np.ascontiguousarray(Wq[:, cols] * scale, dtype=f32).astype(bf),
            "wk": np.ascontiguousarray(Wk[:, cols], dtype=f32).astype(bf),
            "wv": np.ascontiguousarray(Wv[:, cols], dtype=f32).astype(bf),
            "bqkv": np.ascontiguousarray(bqkv, dtype=f32),
            "wo": np.ascontiguousarray(Wo[cols, :], dtype=f32).astype(bf),
            "bias": bias_c,
            "onesv": onesv.astype(bf),
            "vscale": vscale,
        })
    return in_maps


def run_spmd(inputs, Wq, bq, Wk, bk, Wv, bv, Wo, bo, trace=False):
    from concourse.bass_utils import run_bass_kernel_spmd

    if "nc" not in _CACHE:
        _CACHE["nc"] = build_nc()
    nc = _CACHE["nc"]
    in_maps = _prepare_in_maps(inputs, Wq, bq, Wk, bk, Wv, bv, Wo, bo)
    res = run_bass_kernel_spmd(nc, in_maps, list(range(N_CORES)), trace=trace)
    acc = np.zeros((NT, E), dtype=np.float64)
    for c in range(N_CORES):
        acc += res.results[c]["out"]
    out = (acc + np.asarray(bo, dtype=np.float64)[None, :]).astype(np.float32)
    return out.reshape(B, S, E), res


def kernel(inputs, Wq, bq, Wk, bk, Wv, bv, Wo, bo):
    out, _ = run_spmd(inputs, Wq, bq, Wk, bk, Wv, bv, Wo, bo, trace=False)
    return out



# revision 13
# speedup vs baseline: 1.2072x; 1.2072x over previous
"""ALiBi multi-head attention on 8 TRN2 NeuronCores.

Problem: B=2, S=2048, E=1024, H=16 heads of D=64, fp32.
  q/k/v = inputs @ W* + b*;  scores = q k^T / sqrt(D) + slope_h * (j - i)
  out = softmax(scores) @ v, heads concat, @ Wo + bo.

Sharding: tensor-parallel over heads - 2 heads per core, both batches on
every core. Each core computes its heads' q/k/v projections, attention,
and a partial output projection (row-parallel Wo); the host sums the 8
partials (bf16) and adds bo.

Key algebra: softmax over j is invariant to per-row shifts, so the whole
ALiBi bias slope*(j - i) reduces (after dropping the -slope*i row shift)
to a per-KEY factor exp(slope*(j - (S-1))) that multiplies column j of
the attention numerator AND denominator identically. We fold that factor
into v (and into the all-ones ride-along column appended to v that
accumulates the softmax denominators through the PV matmul) at host prep
time. The device then computes plain exp(qk*scale): NO bias operand, no
row-max pass, no row-sum pass, and every Exp activation is head-agnostic.

QK matmuls contract only K=64 (head dim), so we run them as PE row-tile
PAIRS: two K=64 matmuls in array halves (partitions 0-63 / 64-127)
execute concurrently (tile_position row groups), doubling QK throughput.
This needs each head's q/k on both partition halves: qT/kTP hold
[headA | headB] and qdup/kdup hold the partition-swapped copy (built with
two SBUF->SBUF DMAs per half while projections still run).

The light head slot ("B", heads 0-7) only processes the last 2 j-chunks:
the ALiBi decay bounds every dropped weight below ~e^-14 of the row sum.
The heavy slot ("A", heads 8-15) runs all 16. Per (b,qi) block: 9 QK
pair-slots, each [128,2,512] PSUM tile -> one 1024-wide Exp -> 2 PV
matmuls. Output projection for block n is deferred into block n+1's QK
phase so the softmax-normalize chain never stalls the PE.

All DMA transfers are batched into single multi-dimensional descriptors
(one trigger per 512-token x-slice, per weight matrix, per output block):
DMA trigger instructions cost ~0.6us each on the Sync engine, which was
the projection-phase bottleneck in v1.

Matmul operands are bf16 (f32 PSUM accumulate). Output partials are
written bf16 and summed f64 on the host. Rel err vs the f32 reference
~5e-3 (dominated by bf16 operand rounding), inside the 2e-2 gate.
"""

import numpy as np

NUM_HEADS = 16
E = 1024
D = 64
B = 2
S = 2048
N_CORES = 8
HL = NUM_HEADS // N_CORES      # heads per core = 2
COLS = HL * D                  # per-core projection width = 128
NT = B * S                     # total tokens = 4096
NJC = S // 128                 # 16 j-chunks per batch
NTC = NT // 128                # 32 global token chunks
NTB = NT // 512                # 8 token blocks for projections
NQI = S // 512                 # 4 qi-blocks per batch
NB_CH = 2                      # j-chunks kept for the light head slot

_CACHE = {}
DEBUG_DUMP = False


def _alibi_slopes():
    x = (2.0 ** 8) ** (1.0 / NUM_HEADS)
    return [1.0 / x ** (i + 1) for i in range(NUM_HEADS)]


def build_nc():
    import concourse.mybir as mybir
    import concourse.tile as tile
    from concourse import bacc
    from concourse.masks import make_identity

    f32 = mybir.dt.float32
    bf16 = mybir.dt.bfloat16
    Exp = mybir.ActivationFunctionType.Exp

    nc = bacc.Bacc("TRN2", target_bir_lowering=False, debug=False,
                   num_devices=N_CORES)

    xt_ext = nc.declare_dram_parameter("xt", [128, 8, NT], bf16, isOutput=False)
    wq_ext = nc.declare_dram_parameter("wq", [128, 8, COLS], bf16, isOutput=False)
    wk_ext = nc.declare_dram_parameter("wk", [128, 8, COLS], bf16, isOutput=False)
    wv_ext = nc.declare_dram_parameter("wv", [128, 8, COLS], bf16, isOutput=False)
    bqkv_ext = nc.declare_dram_parameter("bqkv", [COLS, 3], f32, isOutput=False)
    wo_ext = nc.declare_dram_parameter("wo", [COLS, E], bf16, isOutput=False)
    onesv_ext = nc.declare_dram_parameter("onesv", [128, NTC, 2], bf16,
                                          isOutput=False)
    vscale_ext = nc.declare_dram_parameter("vscale", [128, NTC, 2], f32,
                                           isOutput=False)
    # out[p, tc4, n, e]: token id = n*512 + tc4*128 + p  (n = b*NQI + qi)
    out_ext = nc.declare_dram_parameter("out", [128, 4, B * NQI, E], bf16,
                                        isOutput=True)
    if DEBUG_DUMP:
        dmp = {
            name: nc.declare_dram_parameter(f"dump_{name}", [128, NT], bf16,
                                            isOutput=True)
            for name in ("qT", "kTP", "qdup", "kdup")
        }
        ctxd_ext = nc.declare_dram_parameter("dump_ctx", [128, B * NQI, 512],
                                             bf16, isOutput=True)
        dend_ext = nc.declare_dram_parameter("dump_den", [2, B * NQI, 512],
                                             mybir.dt.float32, isOutput=True)
        vsbd_ext = nc.declare_dram_parameter("dump_vsb", [128, NTC, 2 * (D + 1)],
                                             bf16, isOutput=True)
        vsc_ext = nc.declare_dram_parameter("dump_vscale", [128, NTC, 2],
                                            mybir.dt.float32, isOutput=True)
        rawc_ext = nc.declare_dram_parameter("dump_rawctx", [4, 2, B * NQI, 512],
                                             mybir.dt.float32, isOutput=True)
        r1d_ext = nc.declare_dram_parameter("dump_r1", [1, 2, B * NQI, 512],
                                            mybir.dt.float32, isOutput=True)
        rbd_ext = nc.declare_dram_parameter("dump_rb", [64, B * NQI, 512],
                                            mybir.dt.float32, isOutput=True)

    from contextlib import ExitStack
    with tile.TileContext(nc) as tc, ExitStack() as stack:
        with (
            tc.tile_pool(name="persist", bufs=1) as pp,
            tc.tile_pool(name="stage", bufs=2) as stp,
            tc.tile_pool(name="exp", bufs=3) as expp,
            tc.tile_pool(name="norm", bufs=3) as nrm,
            tc.tile_pool(name="ctx", bufs=2) as ctxp,
            tc.tile_pool(name="outp", bufs=2) as outp,
        ):
            # ---- persistent tiles ----
            wq_sb = pp.tile([128, 8, COLS], bf16, tag="wq")
            wk_sb = pp.tile([128, 8, COLS], bf16, tag="wk")
            wv_sb = pp.tile([128, 8, COLS], bf16, tag="wv")
            xt_full = pp.tile([128, 8, NT], bf16, tag="xt_full")
            bqkv_sb = pp.tile([128, 3], f32, tag="bqkv")
            wo_sb = pp.tile([128, E], bf16, tag="wo")
            vscale_sb = pp.tile([128, NTC, 2], f32, tag="vscale")
            ident = pp.tile([128, 128], bf16, tag="ident")
            qT = pp.tile([128, NT], bf16, tag="qT")
            kTP = pp.tile([128, NT], bf16, tag="kTP")
            qdup = pp.tile([128, NT], bf16, tag="qdup")
            kdup = pp.tile([128, NT], bf16, tag="kdup")
            v_sb = pp.tile([128, NTC, 2 * (D + 1)], bf16, tag="v")
            sv_stage = pp.tile([128, NTC, 2], bf16, tag="sv_stage")

            # ---- load constants: one batched DMA trigger per tensor ----
            nc.sync.dma_start(out=wq_sb[:], in_=wq_ext[:])
            nc.sync.dma_start(out=xt_full[:, :, 0:512], in_=xt_ext[:, :, 0:512])
            nc.sync.dma_start(out=bqkv_sb[:], in_=bqkv_ext[:])
            nc.sync.dma_start(out=wk_sb[:], in_=wk_ext[:])
            nc.sync.dma_start(out=wv_sb[:], in_=wv_ext[:])
            nc.sync.dma_start(out=wo_sb[:], in_=wo_ext[:])
            nc.sync.dma_start(out=vscale_sb[:], in_=vscale_ext[:])
            nc.sync.dma_start(out=sv_stage[:], in_=onesv_ext[:])
            make_identity(nc, ident[:])
            # denominator ride-along columns (pre-scaled ones)
            nc.vector.tensor_copy(v_sb[:, :, D:D + 1], sv_stage[:, :, 0:1])
            nc.vector.tensor_copy(v_sb[:, :, 2 * D + 1:2 * D + 2],
                                  sv_stage[:, :, 1:2])

            psQK = stack.enter_context(
                tc.tile_pool(name="psQK", bufs=2, space="PSUM"))

            # ---- projections ----
            with (
                tc.tile_pool(name="psA", bufs=2, space="PSUM") as psA,
                tc.tile_pool(name="psT", bufs=2, space="PSUM") as psT,
            ):
                with nc.named_scope("proj"):
                    for tb in range(NTB):
                        ts = slice(tb * 512, (tb + 1) * 512)
                        if tb < NTB - 1:
                            ts2 = slice((tb + 1) * 512, (tb + 2) * 512)
                            nc.sync.dma_start(out=xt_full[:, :, ts2],
                                              in_=xt_ext[:, :, ts2])
                        for pi, (w_sb, kind) in enumerate(
                            ((wq_sb, "q"), (wk_sb, "k"), (wv_sb, "v"))
                        ):
                            ps = psA.tile([128, 512], f32, tag="pa", name="ps")
                            for kc in range(8):
                                nc.tensor.matmul(
                                    ps[:],
                                    w_sb[:, kc, :],
                                    xt_full[:, kc, ts],
                                    start=(kc == 0), stop=(kc == 7))
                            if kind == "q":
                                nc.vector.tensor_scalar_add(
                                    qT[:, ts], ps[:], bqkv_sb[:, pi:pi + 1])
                            elif kind == "k":
                                nc.vector.tensor_scalar_add(
                                    kTP[:, ts], ps[:], bqkv_sb[:, pi:pi + 1])
                            else:
                                vT_t = stp.tile([128, 512], bf16, tag="vT")
                                nc.vector.tensor_scalar_add(
                                    vT_t[:], ps[:], bqkv_sb[:, pi:pi + 1])
                                for i in range(4):
                                    t = tb * 4 + i
                                    pt = psT.tile([128, 128], bf16, tag="pt",
                                                  name="pt")
                                    nc.tensor.transpose(
                                        pt[:], vT_t[:, i * 128:(i + 1) * 128],
                                        ident[:])
                                    # fold the full ALiBi key factor into v
                                    nc.vector.tensor_scalar_mul(
                                        v_sb[:, t, 0:D],
                                        pt[:, 0:D], vscale_sb[:, t, 0:1])
                                    nc.vector.tensor_scalar_mul(
                                        v_sb[:, t, D + 1:2 * D + 1],
                                        pt[:, D:2 * D], vscale_sb[:, t, 1:2])
                        if tb == 3 or tb == NTB - 1:
                            # partition-swapped dup of q/k for row-tiled QK
                            hs = slice(0, 2048) if tb == 3 else slice(2048, NT)
                            for src, dst in ((qT, qdup), (kTP, kdup)):
                                nc.sync.dma_start(out=dst[64:128, hs],
                                                  in_=src[0:64, hs])
                                nc.sync.dma_start(out=dst[0:64, hs],
                                                  in_=src[64:128, hs])
                    if DEBUG_DUMP:
                        for name, t in (("qT", qT), ("kTP", kTP),
                                        ("qdup", qdup), ("kdup", kdup)):
                            nc.sync.dma_start(out=dmp[name][:], in_=t[:])
                        nc.sync.dma_start(out=vsbd_ext[:], in_=v_sb[:])
                        nc.sync.dma_start(out=vsc_ext[:], in_=vscale_sb[:])

            # ---- attention ----
            with (
                tc.tile_pool(name="psC", bufs=1, space="PSUM") as psC,
                tc.tile_pool(name="psW", bufs=2, space="PSUM") as psW,
            ):
                def emit_wo(ctx_sb, n):
                    o_big = outp.tile([128, 4, E], bf16, tag="out",
                                      name="o_big")
                    for tc4 in range(4):
                        for ec in range(2):
                            wo_ps = psW.tile([128, 512], f32, tag="wo",
                                             name="wo_ps")
                            nc.tensor.matmul(
                                wo_ps[:],
                                ctx_sb[:, tc4 * 128:(tc4 + 1) * 128],
                                wo_sb[:, ec * 512:(ec + 1) * 512],
                                start=True, stop=True)
                            nc.vector.tensor_copy(
                                o_big[:, tc4, ec * 512:(ec + 1) * 512],
                                wo_ps[:])
                    nc.sync.dma_start(out=out_ext[:, :, n, :], in_=o_big[:])

                with nc.named_scope("attn"):
                    prev = None  # (ctx_sb, n) awaiting output projection
                    for n in range(B * NQI):
                        b, qi = divmod(n, NQI)
                        boff = b * S
                        qs = slice(boff + qi * 512, boff + qi * 512 + 512)
                        ctx_ps = [psC.tile([D + 1, 512], f32, tag=f"ctx{h}",
                                           name=f"ctx{h}")
                                  for h in range(HL)]
                        ctx_sb = ctxp.tile([128, 512], bf16, tag="ctx_sb")

                        def emit_norm(h, ctx_ps=ctx_ps, ctx_sb=ctx_sb, n=n):
                            if DEBUG_DUMP:
                                dd = nrm.tile([1, 512], f32, tag=f"dd{h}",
                                              name=f"dd{h}")
                                nc.vector.tensor_copy(
                                    dd[:], ctx_ps[h][D:D + 1, :])
                                nc.sync.dma_start(
                                    out=dend_ext[h:h + 1, n, :], in_=dd[:])
                                rc = nrm.tile([4, 512], f32, tag=f"rc{h}",
                                              name=f"rc{h}")
                                nc.scalar.copy(rc[:], ctx_ps[h][0:4, :])
                                nc.sync.dma_start(
                                    out=rawc_ext[:, h, n, :], in_=rc[:])
                            s0 = nrm.tile([1, 512], f32, tag=f"s0{h}",
                                          name=f"s0{h}")
                            # plain DVE copy handles the partition-64 PSUM
                            # read; the custom-DVE reciprocal op does NOT
                            # (it reads at the output's base partition).
                            nc.vector.tensor_copy(
                                s0[:], ctx_ps[h][D:D + 1, :])
                            r1 = nrm.tile([1, 512], f32, tag=f"r1{h}",
                                          name=f"r1{h}")
                            nc.vector.reciprocal_approx_fast(r1[:], s0[:])
                            rb = nrm.tile([D, 512], f32, tag="rb")
                            nc.gpsimd.partition_broadcast(rb[:], r1[:])
                            if DEBUG_DUMP:
                                nc.sync.dma_start(
                                    out=r1d_ext[:, h, n, :], in_=r1[:])
                                if h == 0:
                                    nc.sync.dma_start(
                                        out=rbd_ext[:, n, :], in_=rb[:])
                            nc.vector.tensor_mul(
                                ctx_sb[h * D:(h + 1) * D, :],
                                ctx_ps[h][0:D, :], rb[:])

                        def emit_pv(item, b=b, ctx_ps=ctx_ps,
                                    emit_norm=emit_norm):
                            h, jl, jh, e_t = item
                            hc = slice(h * (D + 1), (h + 1) * (D + 1))
                            first = 0 if h == 0 else NJC - NB_CH
                            for u, jc in ((0, jl), (1, jh)):
                                t = b * NJC + jc
                                nc.tensor.matmul(
                                    ctx_ps[h][:],
                                    v_sb[:, t, hc],
                                    e_t[:, u, :],
                                    start=(jc == first),
                                    stop=(jc == NJC - 1))
                            if h == 1:
                                emit_norm(1)

                        # 9 pair-slots: (head-slot, lo-chunk, hi-chunk).
                        # B first so its normalize hides under A's slots.
                        sched = ([(1, NJC - 2, NJC - 1)]
                                 + [(0, 2 * g, 2 * g + 1)
                                    for g in range(NJC // 2)])
                        pend = []     # exp'd slots awaiting PV
                        for s, (h, jl, jh) in enumerate(sched):
                            qk_t = psQK.tile([128, 2, 512], f32,
                                             tag="qk", name="qk")
                            for u, jc in ((0, jl), (1, jh)):
                                j0 = boff + jc * 128
                                # lo-half MM streams array rows 0-63,
                                # hi-half rows 64-127: concurrent row tiles
                                if h == 0:
                                    ksrc, qsrc = ((kTP, qT) if u == 0
                                                  else (kdup, qdup))
                                else:
                                    ksrc, qsrc = ((kdup, qdup) if u == 0
                                                  else (kTP, qT))
                                lo = 0 if u == 0 else 64
                                nc.tensor.matmul(
                                    qk_t[:, u, :],
                                    ksrc[lo:lo + 64, j0:j0 + 128],
                                    qsrc[lo:lo + 64, qs],
                                    start=True, stop=True)
                            e_t = expp.tile([128, 2, 512], bf16, tag="exp")
                            nc.scalar.activation(e_t[:], qk_t[:], Exp)
                            pend.append((h, jl, jh, e_t))
                            if s == 1 and prev is not None:
                                emit_wo(*prev)
                                prev = None
                            if s >= 2:
                                emit_pv(pend.pop(0))
                        for item in pend:
                            emit_pv(item)
                        emit_norm(0)
                        if DEBUG_DUMP:
                            nc.sync.dma_start(out=ctxd_ext[:, n, :],
                                              in_=ctx_sb[:])
                        prev = (ctx_sb, n)
                    emit_wo(*prev)
    nc.compile()
    return nc


def _prepare_in_maps(inputs, Wq, bq, Wk, bk, Wv, bv, Wo, bo):
    import ml_dtypes
    f32 = np.float32
    bf = ml_dtypes.bfloat16
    X = np.asarray(inputs, dtype=f32).reshape(NT, E)
    # xt[p, kc, t] = X[t, kc*128 + p]
    xt = np.ascontiguousarray(
        X.T.reshape(8, 128, NT).transpose(1, 0, 2)).astype(bf)
    slopes = _alibi_slopes()
    scale = 1.0 / np.sqrt(D)

    def wsplit(W):
        # [p, kc, col] = W[kc*128 + p, col]
        return np.ascontiguousarray(
            np.asarray(W, dtype=f32).reshape(8, 128, -1).transpose(1, 0, 2)
        ).astype(bf)

    p = np.arange(128, dtype=np.float64)
    in_maps = []
    for c in range(N_CORES):
        # slot 0 = heavy head (small slope, all chunks), slot 1 = light head
        heads = (8 + c, c)
        cols = np.concatenate([np.arange(h * D, (h + 1) * D) for h in heads])
        bqkv = np.stack([bq[cols] * scale, bk[cols], bv[cols]], axis=1)
        vscale = np.zeros((128, NTC, 2), dtype=f32)
        for l, hh in enumerate(heads):
            sl = slopes[hh]
            for t in range(NTC):
                jc = t % NJC
                # full ALiBi key factor exp(slope * (j - (S-1))), j = 128*jc+p
                vscale[:, t, l] = np.exp(sl * (128.0 * jc + p - (S - 1.0)))
        in_maps.append({
            "xt": xt,
            "wq": wsplit(Wq[:, cols] * scale),
            "wk": wsplit(Wk[:, cols]),
            "wv": wsplit(Wv[:, cols]),
            "bqkv": np.ascontiguousarray(bqkv, dtype=f32),
            "wo": np.ascontiguousarray(Wo[cols, :], dtype=f32).astype(bf),
            "onesv": vscale.astype(bf),
            "vscale": vscale,
        })
    return in_maps


def run_spmd(inputs, Wq, bq, Wk, bk, Wv, bv, Wo, bo, trace=False):
    from concourse.bass_utils import run_bass_kernel_spmd

    if "nc" not in _CACHE:
        _CACHE["nc"] = build_nc()
    nc = _CACHE["nc"]
    in_maps = _prepare_in_maps(inputs, Wq, bq, Wk, bk, Wv, bv, Wo, bo)
    res = run_bass_kernel_spmd(nc, in_maps, list(range(N_CORES)), trace=trace)
    acc = np.zeros((NT, E), dtype=np.float64)
    for c in range(N_CORES):
        # out[p, tc4, n, e] -> token = n*512 + tc4*128 + p
        arr = np.asarray(res.results[c]["out"], dtype=np.float64)
        acc += arr.transpose(2, 1, 0, 3).reshape(NT, E)
    out = (acc + np.asarray(bo, dtype=np.float64)[None, :]).astype(np.float32)
    return out.reshape(B, S, E), res


def kernel(inputs, Wq, bq, Wk, bk, Wv, bv, Wo, bo):
    out, _ = run_spmd(inputs, Wq, bq, Wk, bk, Wv, bv, Wo, bo, trace=False)
    return out
